# revision 1
# baseline (speedup 1.0000x reference)
"""AttentionGRU Trainium2 kernel: 8-core data-parallel over batch.

Reference computation (per example):
  xg = x @ w_ih.T + b_ih                      # hoisted input GEMM, [S, 3H]
  per step t: hg = h @ w_hh.T + b_hh
              r = sigmoid(xg_r + hg_r); z = sigmoid(xg_z + hg_z)
              n = tanh(xg_n + r * hg_n); h = (1-z)*n + z*h
  logits = out @ w_attn.T (+b_attn, softmax-invariant -> dropped)
  attn = softmax over seq; context = sum(attn * out); y = context @ w_fc.T + b_fc

Device layout (per core, B=32 examples):
  - h kept as [H=64 partitions, b free]; gates as [gate, b]. The recurrence
    is latency-bound (~2.3us/step chain of 7 instructions); one fused
    32-example chain beat dual 16-example chains because Tile's in-order
    sequencers head-of-line block on multi-producer waits and DVE pays
    ~170ns fixed cost per op.
  - Phase 1: xT (host-pretransposed, t-major tokens) [128(i), B*S] ->
    xg[g, t*B+b] via two constant stationaries; biases folded via
    per-partition bias ops; xg stored bf16 in four per-128-step tiles. The
    first 8 GEMM chunks are emitted up front, the remaining 24 interleaved
    into the recurrence emission (2 per 16 steps) and the PSUM pool is
    shared across phases 1+2, so the input GEMM tail overlaps the
    recurrence instead of serializing at the pool/tile boundaries.
  - Phase 2 per step: PE computes w_hh gates (b_hh_n via ones-row-augmented
    h) + identity-accumulate of xg into PSUM; ACT sigmoid straight from
    PSUM; DVE p = r*hn, q = p + xn; ACT tanh; DVE m1 = (1-z)*n,
    h' = m1 + z*h, with u = 1-z and m2 = z*h computed inside the tanh
    window (m2 reads h in DVE program order before h's update, so the WAR
    needs no semaphore). z is moved to partitions 0:63 by an identity-slice
    matmul (walrus requires equal SBUF input base partitions). Logits
    l_t = w_attn . h_t via a 1-column matmul into a PSUM strip flushed to
    DRAM every 32 steps; h_t history rows (gpsimd snapshot) DMA'd to
    [t, h*32+b] tiles.
  - Phase 3: softmax on [b, t], PE-transpose of attn, per-example
    accumulated matmuls for context, final FC with bias via an augmented
    ones-row.
"""

import sys

sys.path.insert(0, "/opt/trn_rl_repo")

import numpy as np

import concourse.bacc as bacc
from concourse.bass import _add_dep_helper
import concourse.tile as tile
from concourse import mybir
from concourse import bass_utils

F32 = mybir.dt.float32
BF16 = mybir.dt.bfloat16
AF = mybir.ActivationFunctionType
ALU = mybir.AluOpType

H = 64
I = 128
G = 3 * H  # 192
C = 2
N_CORES = 8
NCH = 1  # independent batch chains per core


def build_program(S: int, B: int = 32, num_devices: int = N_CORES):
    TOK = B * S
    BC = B // NCH  # examples per chain
    nc = bacc.Bacc(
        "TRN2", target_bir_lowering=False, debug=False, num_devices=num_devices
    )

    xT_d = nc.dram_tensor("xT", [I, TOK], F32, kind="ExternalInput")
    w_ihT_d = nc.dram_tensor("w_ihT", [I, G], F32, kind="ExternalInput")
    w_hhT_d = nc.dram_tensor("w_hhT_aug", [H + 1, G], F32, kind="ExternalInput")
    bias_rz_d = nc.dram_tensor("bias_rz", [2 * H, 1], F32, kind="ExternalInput")
    bias_n_d = nc.dram_tensor("bias_n", [H, 1], F32, kind="ExternalInput")
    ident_d = nc.dram_tensor("ident", [128, 128], F32, kind="ExternalInput")
    wattn_d = nc.dram_tensor("w_attn_col", [H, 1], F32, kind="ExternalInput")
    wfc_d = nc.dram_tensor("w_fcT_aug", [H + 1, C], F32, kind="ExternalInput")
    y_d = nc.dram_tensor("y", [B, C], F32, kind="ExternalOutput")
    l_ds = [
        nc.dram_tensor(f"l_scratch{ch}", [1, BC * S], F32, kind="Internal")
        for ch in range(NCH)
    ]

    n_tchunk = (S + 127) // 128  # 128-step history chunks
    assert S % 32 == 0

    with tile.TileContext(nc) as tc:
        with (
            tc.tile_pool(name="const", bufs=1) as const,
            tc.tile_pool(name="share", bufs=1) as share,
            tc.tile_pool(name="xg", bufs=1) as xgp,
            tc.tile_pool(name="sm", bufs=1) as smp,
            tc.tile_pool(name="step", bufs=4) as sp,
            tc.tile_pool(name="snap", bufs=4) as snapp,
            tc.tile_pool(name="p3", bufs=1) as p3,
        ):
            # ---- constants ----
            w_ihT = const.tile([I, G], F32)
            nc.sync.dma_start(out=w_ihT, in_=w_ihT_d.ap())
            w_hhT = const.tile([H + 1, G], F32)
            nc.sync.dma_start(out=w_hhT, in_=w_hhT_d.ap())
            bias_rz = const.tile([2 * H, 1], F32)
            nc.sync.dma_start(out=bias_rz, in_=bias_rz_d.ap())
            bias_n = const.tile([H, 1], F32)
            nc.sync.dma_start(out=bias_n, in_=bias_n_d.ap())
            ident = const.tile([128, 128], F32)
            nc.sync.dma_start(out=ident, in_=ident_d.ap())
            wattn = const.tile([H, 1], F32)
            nc.sync.dma_start(out=wattn, in_=wattn_d.ap())
            wfc = const.tile([H + 1, C], F32)
            nc.sync.dma_start(out=wfc, in_=wfc_d.ap())
            ident_bf = const.tile([128, 128], BF16)
            nc.vector.tensor_copy(ident_bf, ident)

            # ---- xT load (shares slot with history later) ----
            xT = share.tile([I, TOK], F32, tag="big")
            n_ld = max(1, TOK // 1024)
            for c in range(n_ld):
                sl = slice(c * (TOK // n_ld), (c + 1) * (TOK // n_ld))
                nc.sync.dma_start(out=xT[:, sl], in_=xT_d.ap()[:, sl])

            # xg split into per-128-step tiles: phase-2 steps in t-chunk c
            # depend only on tile c, so the recurrence starts as soon as the
            # first GEMM chunk lands instead of after the whole input GEMM
            n_tch = (S + 127) // 128
            TCH = TOK // n_tch
            xg_rz_t = [
                xgp.tile([2 * H, TCH], BF16, name=f"xg_rz{c}") for c in range(n_tch)
            ]
            xg_n_t = [
                xgp.tile([H, TCH], BF16, name=f"xg_n{c}") for c in range(n_tch)
            ]

            # ---- phase 1: input GEMM ----
            n_ck = TOK // 512
            psp12_cm = tc.tile_pool(name="ps12", bufs=1, space="PSUM")
            psp1 = psp12_cm.__enter__()
            ck_per_tile = n_ck // n_tch

            def emit_gemm_chunk(c):
                sl = slice(c * 512, (c + 1) * 512)
                ps_rz1 = psp1.tile(
                    [2 * H, 512], F32, tag="rz", bufs=1, name=f"ps_rz1_{c}"
                )
                nc.tensor.matmul(
                    ps_rz1, lhsT=w_ihT[:, 0 : 2 * H], rhs=xT[:, sl],
                    start=True, stop=True,
                )
                ps_n1 = psp1.tile([H, 512], F32, tag="n", bufs=1, name=f"ps_n1_{c}")
                nc.tensor.matmul(
                    ps_n1, lhsT=w_ihT[:, 2 * H : G], rhs=xT[:, sl],
                    start=True, stop=True,
                )
                dst = slice((c % ck_per_tile) * 512, (c % ck_per_tile + 1) * 512)
                nc.scalar.activation(
                    xg_rz_t[c // ck_per_tile][:, dst], ps_rz1, AF.Identity,
                    bias=bias_rz, scale=1.0,
                )
                nc.vector.tensor_scalar_add(
                    xg_n_t[c // ck_per_tile][:, dst], ps_n1, bias_n
                )

            # head start: first t-chunk of xg up front; the rest of the input
            # GEMM is emitted interleaved into the recurrence (2 chunks per 16
            # steps) so it rides the recurrence's idle engine slots
            next_chunk = ck_per_tile
            for c in range(ck_per_tile):
                emit_gemm_chunk(c)

            # xg views per tile: [gate, t_local, chain, b] (t-major tokens)
            xg_rz_v = [
                x.rearrange("g (s c b) -> g s c b", c=NCH, s=S // n_tch)
                for x in xg_rz_t
            ]
            xg_n_v = [
                x.rearrange("g (s c b) -> g s c b", c=NCH, s=S // n_tch)
                for x in xg_n_t
            ]

            # ---- phase 2: recurrence (NCH interleaved chains) ----
            # history rows: [t_mod, chunk, chain*1024 + h*BC + b]
            hist = xgp.tile([128, n_tchunk, NCH, H * BC], F32)
            h_aug = [smp.tile([H + 1, BC], F32, tag=f"h{ch}", name=f"h_aug{ch}") for ch in range(NCH)]
            for ch in range(NCH):
                nc.vector.memset(h_aug[ch][0:H], 0.0)
                nc.vector.memset(h_aug[ch][H : H + 1], 1.0)

            psp2 = psp1  # same pool: no pool-boundary barrier between phases
            if True:
                ps_l = [None] * NCH
                def emit_logits(ch):
                    # deferred one iteration and emitted AFTER the next step's
                    # front matmuls so it never head-blocks them in the PE
                    # queue. 16-step single-bank PSUM strips with bufs=2 so a
                    # new block's PE writes and the old block's ACT flush-read
                    # land in different banks (P10 hazard hardening).
                    s = pend_l[ch]
                    if s is None:
                        return
                    if s % 16 == 0:
                        ps_l[ch] = psp2.tile(
                            [1, BC * 16], F32, tag=f"psl{ch}", name=f"ps_l{ch}",
                            bufs=2,
                        )
                    nc.tensor.matmul(
                        ps_l[ch][:, (s % 16) * BC : (s % 16 + 1) * BC],
                        lhsT=wattn, rhs=h_aug[ch][0:H], start=True, stop=True,
                    )
                    if s % 16 == 15:
                        blk = s // 16
                        l_sb = sp.tile(
                            [1, BC * 16], F32, tag=f"lsb{ch}", name=f"l_sb{ch}"
                        )
                        nc.scalar.activation(l_sb, ps_l[ch], AF.Identity)
                        nc.sync.dma_start(
                            out=l_ds[ch].ap()[
                                :, blk * BC * 16 : (blk + 1) * BC * 16
                            ],
                            in_=l_sb,
                        )
                    pend_l[ch] = None

                pend_l = [None] * NCH
                for t in range(S):
                    ps_rz, ps_n, ps_z, rz = [], [], [], []
                    # PE front: same stationary back-to-back across chains
                    for ch in range(NCH):
                        # xg-accumulate first: it has no dependency on h, so
                        # the PE runs it in the previous step's idle window and
                        # only the 53ns W.h matmul sits between hadd and sigmoid
                        ps_rz.append(psp2.tile([2 * H, BC], F32, tag=f"psrz{ch}", name=f"ps_rz{ch}", bufs=2))
                        nc.tensor.matmul(
                            ps_rz[ch], lhsT=ident_bf, rhs=xg_rz_v[t // (S // n_tch)][:, t % (S // n_tch), ch, :],
                            start=True, stop=False,
                        )
                    for ch in range(NCH):
                        nc.tensor.matmul(
                            ps_rz[ch], lhsT=w_hhT[:, 0 : 2 * H], rhs=h_aug[ch],
                            start=False, stop=True,
                        )
                    for ch in range(NCH):
                        ps_n.append(psp2.tile([H, BC], F32, tag=f"psn{ch}", name=f"ps_n{ch}"))
                        nc.tensor.matmul(
                            ps_n[ch], lhsT=w_hhT[:, 2 * H : G], rhs=h_aug[ch],
                            start=True, stop=True,
                        )
                    for ch in range(NCH):
                        emit_logits(ch)
                    for ch in range(NCH):
                        r_t = sp.tile([2 * H, BC], F32, tag=f"rz{ch}")
                        nc.scalar.activation(r_t, ps_rz[ch], AF.Sigmoid)
                        rz.append(r_t)
                    for ch in range(NCH):
                        ps_z.append(psp2.tile([H, BC], F32, tag=f"psz{ch}", name=f"ps_z{ch}"))
                        nc.tensor.matmul(
                            ps_z[ch], lhsT=ident[:, H : 2 * H], rhs=rz[ch],
                            start=True, stop=True,
                        )
                    p, q, nt, i_q = [], [], [], []
                    for ch in range(NCH):
                        p.append(sp.tile([H, BC], F32, tag=f"p{ch}", name=f"p{ch}"))
                        nc.vector.tensor_mul(p[ch], rz[ch][0:H], ps_n[ch])
                        q.append(sp.tile([H, BC], F32, tag=f"q{ch}", name=f"q{ch}"))
                        i_q.append(nc.vector.tensor_add(q[ch], p[ch], xg_n_v[t // (S // n_tch)][:, t % (S // n_tch), ch, :]))
                    for ch in range(NCH):
                        nt.append(sp.tile([H, BC], F32, tag=f"nt{ch}", name=f"nt{ch}"))
                        nc.scalar.activation(nt[ch], q[ch], AF.Tanh)
                    for ch in range(NCH):
                        # u = 1-z and m2 = z*h_prev run in the tanh window
                        # (m2 reads h_aug in DVE program order before hadd's
                        # write, so the WAR needs no semaphore); only
                        # m1 = u*n and h' = m1+m2 sit after tanh on the chain
                        u = sp.tile([H, BC], F32, tag=f"u{ch}", name=f"u{ch}")
                        i_u = nc.vector.tensor_scalar(
                            u, ps_z[ch], -1.0, 1.0, op0=ALU.mult, op1=ALU.add
                        )
                        # keep the off-chain u/m2 behind q in the DVE stream so
                        # they fill the tanh window instead of delaying it
                        _add_dep_helper(i_u.ins, i_q[ch].ins, sync=False,
                                        reason="order u after q")
                        m2 = sp.tile([H, BC], F32, tag=f"m2{ch}", name=f"m2{ch}")
                        i_m2 = nc.vector.tensor_mul(m2, ps_z[ch], h_aug[ch][0:H])
                        _add_dep_helper(i_m2.ins, i_u.ins, sync=False,
                                        reason="order m2 after u")
                        m1 = sp.tile([H, BC], F32, tag=f"m1{ch}", name=f"m1{ch}")
                        nc.vector.tensor_mul(m1, u, nt[ch])
                        nc.vector.tensor_add(h_aug[ch][0:H], m1, m2)
                    for ch in range(NCH):
                        # snapshot h_t (feeds history DMA, logits, next d)
                        hs = snapp.tile([H, BC], F32, tag=f"hs{ch}")
                        nc.gpsimd.tensor_copy(hs, h_aug[ch][0:H])
                        nc.sync.dma_start(
                            out=hist[t % 128 : t % 128 + 1, t // 128, ch, :],
                            in_=hs,
                        )
                        pend_l[ch] = t
                    if t % 16 == 15:
                        for _ in range(2):
                            if next_chunk < n_ck:
                                emit_gemm_chunk(next_chunk)
                                next_chunk += 1

                for ch in range(NCH):
                    emit_logits(ch)

            psp12_cm.__exit__(None, None, None)

            # ---- phase 3: softmax + context + fc ----
            with tc.tile_pool(name="ps3", bufs=2, space="PSUM") as psp3:
                l_bt = p3.tile([B, S], F32)
                for ch in range(NCH):
                    nc.sync.dma_start(
                        out=l_bt[ch * BC : (ch + 1) * BC],
                        in_=l_ds[ch].ap().rearrange("o (s b) -> (o b) s", b=BC),
                    )
                mx = p3.tile([B, 1], F32)
                nc.vector.reduce_max(mx, l_bt, axis=mybir.AxisListType.X, negate=True)
                e_bt = p3.tile([B, S], F32)
                ssum = p3.tile([B, 1], F32)
                nc.scalar.activation(
                    e_bt, l_bt, AF.Exp, bias=mx, scale=1.0, accum_out=ssum
                )
                rinv = p3.tile([B, 1], F32)
                nc.vector.reciprocal(rinv, ssum)
                attn = p3.tile([B, S], F32)
                nc.vector.tensor_scalar_mul(attn, e_bt, rinv)

                attn_tb = []
                for c in range(n_tchunk):
                    ps_tr = psp3.tile([128, B], F32, tag="pstr")
                    nc.tensor.transpose(
                        ps_tr, attn[:, c * 128 : (c + 1) * 128], ident[0:B, 0:B]
                    )
                    a_tb = p3.tile([128, B], F32, tag=f"atb{c}")
                    nc.vector.tensor_copy(a_tb, ps_tr)
                    attn_tb.append(a_tb)

                hist_v = hist.rearrange("p c ch (h b) -> p c ch h b", b=BC)
                ctx_ps = psp3.tile([H, B], F32, tag="ctx")
                for b in range(B):
                    ch, b16 = b // BC, b % BC
                    for c in range(n_tchunk):
                        nc.tensor.matmul(
                            ctx_ps[:, b : b + 1],
                            lhsT=hist_v[:, c, ch, :, b16],
                            rhs=attn_tb[c][:, b : b + 1],
                            start=(c == 0),
                            stop=(c == n_tchunk - 1),
                        )
                ctx_aug = p3.tile([H + 1, B], F32)
                nc.vector.memset(ctx_aug[H : H + 1], 1.0)
                nc.vector.tensor_copy(ctx_aug[0:H], ctx_ps)
                y_ps = psp3.tile([C, B], F32, tag="y")
                nc.tensor.matmul(y_ps, lhsT=wfc, rhs=ctx_aug, start=True, stop=True)
                y_sb = p3.tile([C, B], F32)
                nc.vector.tensor_copy(y_sb, y_ps)
                nc.sync.dma_start(out=y_d.ap().rearrange("b c -> c b"), in_=y_sb)

    nc.compile()
    return nc


def prep_core_inputs(x_shard, w_ih, w_hh, b_ih, b_hh, w_attn, w_fc, b_fc):
    """Build the per-core in_map from a [B, S, I] f32 shard + full params."""
    B, S, I_ = x_shard.shape
    # t-major token order [i, t*B + b]: phase-2's step-t slice is contiguous
    # and phase 1 produces early timesteps first (lets the recurrence start
    # while the input GEMM tail is still running)
    xT = np.ascontiguousarray(
        x_shard.transpose(2, 1, 0).reshape(I_, B * S), dtype=np.float32
    )
    w_hhT_aug = np.zeros((H + 1, G), dtype=np.float32)
    w_hhT_aug[0:H, :] = w_hh.T
    w_hhT_aug[H, 2 * H : G] = b_hh[2 * H : G]  # b_hh_n via ones-row
    bias_rz = (b_ih[0 : 2 * H] + b_hh[0 : 2 * H]).reshape(2 * H, 1)
    bias_n = b_ih[2 * H : G].reshape(H, 1)
    w_fcT_aug = np.zeros((H + 1, C), dtype=np.float32)
    w_fcT_aug[0:H, :] = w_fc.T
    w_fcT_aug[H, :] = b_fc
    return {
        "xT": xT,
        "w_ihT": np.ascontiguousarray(w_ih.T, dtype=np.float32),
        "w_hhT_aug": w_hhT_aug,
        "bias_rz": np.ascontiguousarray(bias_rz, dtype=np.float32),
        "bias_n": np.ascontiguousarray(bias_n, dtype=np.float32),
        "ident": np.eye(128, dtype=np.float32),
        "w_attn_col": np.ascontiguousarray(w_attn.T, dtype=np.float32),
        "w_fcT_aug": w_fcT_aug,
    }


_NC_CACHE = {}


def kernel(x, w_ih, w_hh, b_ih, b_hh, w_attn, b_attn, w_fc, b_fc):
    x = np.asarray(x, dtype=np.float32)
    w_ih = np.asarray(w_ih, dtype=np.float32)
    w_hh = np.asarray(w_hh, dtype=np.float32)
    b_ih = np.asarray(b_ih, dtype=np.float32)
    b_hh = np.asarray(b_hh, dtype=np.float32)
    w_attn = np.asarray(w_attn, dtype=np.float32)
    w_fc = np.asarray(w_fc, dtype=np.float32)
    b_fc = np.asarray(b_fc, dtype=np.float32)

    Bfull, S, _ = x.shape
    B = Bfull // N_CORES
    key = (S, B)
    if key not in _NC_CACHE:
        _NC_CACHE[key] = build_program(S, B, num_devices=N_CORES)
    nc = _NC_CACHE[key]

    in_maps = []
    for c in range(N_CORES):
        shard = x[c * B : (c + 1) * B]
        in_maps.append(
            prep_core_inputs(shard, w_ih, w_hh, b_ih, b_hh, w_attn, w_fc, b_fc)
        )
    res = bass_utils.run_bass_kernel_spmd(nc, in_maps, core_ids=list(range(N_CORES)))
    out = np.concatenate([res.results[c]["y"] for c in range(N_CORES)], axis=0)
    return out.astype(np.float32)



# revision 3
# speedup vs baseline: 1.2698x; 1.2698x over previous
"""AttentionGRU Trainium2 kernel v3: scan-fused recurrence, 8-core batch-parallel.

Per-step chain (5 links, was 7):
  mm1 (W_rz·h into psRZ, accumulating onto phase-1's xg) ->
  sigma (F2 odds = [u; r], z-preact negated so sigma gives u=1-z directly) ->
  X-scan: tensor_tensor_scan(F2[64:128]=[0|r], ps2win=[hn_b|xn]) pairs ->
     even: 0*state+hn_b ; odd: r*hn_b+xn = q ->
  tanh (q odds -> nt at Y1 evens, bias=b_in) ->
  Y-scan: scan(F2[0:64]=[0|u], Y1=[nt|m2]) -> odd: u*nt+m2 = h'
w1=u*h, m2=h-w1 fill the tanh window on DVE; Pool copies h' odds into the
contiguous hist slot for the attention; 3 matmuls/step (W_rz, W_n_aug, w_attn).
b_hn rides a prefilled ones-row (h2 row 64 odds = 1, s2_aug row 64 = b_hn).
Attention: e^l = sigma(l)/sigma(-l) (in-table), Pool broadcast + multiply,
DVE reduces, PSUM-accumulated ctx/sum_e across all blocks.
"""

import sys

sys.path.insert(0, "/opt/trn_rl_repo")

import numpy as np

import concourse.bacc as bacc
import concourse.tile as tile
from concourse import mybir
from concourse import bass_utils

F32 = mybir.dt.float32
BF16 = mybir.dt.bfloat16
AF = mybir.ActivationFunctionType
ALU = mybir.AluOpType

H = 64
I = 128
C = 2
N_CORES = 8


def build_program(S: int, B: int = 32, num_devices: int = N_CORES):
    TOK = B * S
    nblk = S // 16
    BLK = 16 * B  # tokens per block
    W2 = 2 * B  # interleaved window width (64)
    nc = bacc.Bacc(
        "TRN2", target_bir_lowering=False, debug=False, num_devices=num_devices
    )

    xT_d = nc.dram_tensor("xT", [I, TOK], BF16, kind="ExternalInput")
    wihrz_d = nc.dram_tensor("wihrz", [I, 2 * H], BF16, kind="ExternalInput")
    wihn_d = nc.dram_tensor("wihn", [I, H], BF16, kind="ExternalInput")
    s1_d = nc.dram_tensor("s1", [H, 2 * H], F32, kind="ExternalInput")
    s2a_d = nc.dram_tensor("s2a", [H + 1, H], F32, kind="ExternalInput")
    wattn_d = nc.dram_tensor("wattn", [H, 1], F32, kind="ExternalInput")
    biasrzn_d = nc.dram_tensor("biasrzn", [2 * H, 1], F32, kind="ExternalInput")
    bin_d = nc.dram_tensor("bin", [2 * H, 1], F32, kind="ExternalInput")
    ones_d = nc.dram_tensor("ones_f", [1, H], F32, kind="ExternalInput")
    ident_d = nc.dram_tensor("ident", [H, H], F32, kind="ExternalInput")
    sfc_d = nc.dram_tensor("sfc", [H, C], F32, kind="ExternalInput")
    bfc_d = nc.dram_tensor("bfc", [C, 1], F32, kind="ExternalInput")
    y_d = nc.dram_tensor("y", [B, C], F32, kind="ExternalOutput")

    with tile.TileContext(nc) as tc:
        with (
            tc.tile_pool(name="const", bufs=1) as const,
            tc.tile_pool(name="big", bufs=1) as bigp,
            tc.tile_pool(name="step", bufs=2) as sp,
            tc.tile_pool(name="attn", bufs=2) as ap_,
            tc.tile_pool(name="ps", bufs=1, space="PSUM") as ps,
        ):
            # ---- constants ----
            wihrz = const.tile([I, 2 * H], BF16)
            nc.sync.dma_start(out=wihrz, in_=wihrz_d.ap())
            wihn = const.tile([I, H], BF16)
            nc.sync.dma_start(out=wihn, in_=wihn_d.ap())
            s1 = const.tile([H, 2 * H], F32)
            nc.sync.dma_start(out=s1, in_=s1_d.ap())
            s2a = const.tile([H + 1, H], F32)
            nc.sync.dma_start(out=s2a, in_=s2a_d.ap())
            wattn = const.tile([H, 1], F32)
            nc.sync.dma_start(out=wattn, in_=wattn_d.ap())
            biasrzn = const.tile([2 * H, 1], F32)
            nc.sync.dma_start(out=biasrzn, in_=biasrzn_d.ap())
            bin_ = const.tile([2 * H, 1], F32)
            nc.sync.dma_start(out=bin_, in_=bin_d.ap())
            ones_f = const.tile([1, H], F32)
            nc.sync.dma_start(out=ones_f, in_=ones_d.ap())
            ident = const.tile([H, H], F32)
            nc.sync.dma_start(out=ident, in_=ident_d.ap())
            sfc = const.tile([H, C], F32)
            nc.sync.dma_start(out=sfc, in_=sfc_d.ap())
            bfc = const.tile([C, 1], F32)
            nc.sync.dma_start(out=bfc, in_=bfc_d.ap())

            # ---- big SBUF tensors ----
            xT = bigp.tile([I, TOK], BF16)
            n_ld = 8
            for cch in range(n_ld):
                sl = slice(cch * (TOK // n_ld), (cch + 1) * (TOK // n_ld))
                nc.sync.dma_start(out=xT[:, sl], in_=xT_d.ap()[:, sl])
            hist = bigp.tile([H, S + 1, B], F32)  # slot t = h_{t-1} (contiguous)
            nc.vector.memset(hist[:, 0, :], 0.0)

            # explicit ping-pong tiles with persistent prefill (evens=0 for F2,
            # ones row for h2) — plain tiles, not tag-rotated, so the prefill
            # survives across steps
            f2_bufs = []
            h2_bufs = []
            for i in range(2):
                f = bigp.tile([2 * H, W2], F32, name=f"F2buf{i}")
                nc.vector.memset(f, 0.0)
                f2_bufs.append(f)
                h = bigp.tile([H + 1, W2], F32, name=f"h2buf{i}")
                nc.vector.memset(h[H : H + 1], 1.0)
                h2_bufs.append(h)
            z2 = bigp.tile([H + 1, W2], F32, name="z2")
            nc.vector.memset(z2, 0.0)
            nc.vector.memset(z2[H : H + 1], 1.0)

            # ---- PSUM tiles ----
            def psrz_tile(k):
                return ps.tile([2 * H, BLK], F32, tag="rz", bufs=2, name=f"psrz{k}")

            def ps2_tile(k):
                # [hn|xn] interleaved windows (rows 64:128) + l strip (row 0)
                return ps.tile(
                    [2 * H, 2 * BLK], F32, tag="p2", bufs=2, name=f"ps2_{k}"
                )

            psacc = ps.tile([2 * H, BLK], F32, tag="acc", bufs=1, name="psacc")[
                :, 0:B
            ]

            psrz_cur = [None, None]
            ps2_cur = [None, None]

            def phase1(k):
                t0 = k * BLK
                pr = psrz_tile(k)
                psrz_cur[k % 2] = pr
                nc.tensor.matmul(
                    pr, lhsT=wihrz, rhs=xT[:, t0 : t0 + BLK], start=True, stop=True
                )
                p2 = ps2_tile(k)
                ps2_cur[k % 2] = p2
                pn = ps.tile([2 * H, BLK], F32, tag="pn", bufs=1, name=f"psn{k}")
                nc.tensor.matmul(
                    pn[H : 2 * H],
                    lhsT=wihn,
                    rhs=xT[:, t0 : t0 + BLK],
                    start=True,
                    stop=True,
                )
                return pn

            def xn_flush(k, pn, half):
                # xn -> odd columns of the interleaved ps2 windows (ACT write,
                # strided PSUM out is fine for the scalar engine)
                p2 = ps2_cur[k % 2]
                ssl = slice(half * 8, (half + 1) * 8)
                p2v = p2[H : 2 * H].rearrange(
                    "p (s b two) -> p s b two", b=B, two=2
                )
                pnv = pn[H : 2 * H].rearrange("p (s b) -> p s b", b=B)
                nc.scalar.activation(p2v[:, ssl, :, 1], pnv[:, ssl, :], AF.Identity)

            pn0 = phase1(0)
            xn_flush(0, pn0, 0)
            xn_flush(0, pn0, 1)

            # attention-block state
            attn_state = {}

            def attn_head(k, j):
                st = attn_state.setdefault(k, {})
                p2 = ps2_cur[k % 2]
                lview = p2[0:1].rearrange("o (s c) -> o s c", c=W2)[:, :, 0:B]
                if j in (1, 2):
                    # ACT: sigma(+l), b-quarters x2 per window
                    if "spos" not in st:
                        st["spos"] = ap_.tile([1, BLK], F32, tag="spos", name="spos")
                    for q in range(2):
                        qtr = (j - 1) * 2 + q
                        bsl = slice(qtr * (B // 4), (qtr + 1) * (B // 4))
                        nc.scalar.activation(
                            st["spos"].rearrange("o (b s) -> o s b", s=16)[:, :, bsl],
                            lview[:, :, bsl],
                            AF.Sigmoid,
                        )
                elif j in (3, 4):
                    if "sneg" not in st:
                        st["sneg"] = ap_.tile([1, BLK], F32, tag="sneg", name="sneg")
                    for q in range(2):
                        qtr = (j - 3) * 2 + q
                        bsl = slice(qtr * (B // 4), (qtr + 1) * (B // 4))
                        nc.scalar.activation(
                            st["sneg"].rearrange("o (b s) -> o s b", s=16)[:, :, bsl],
                            lview[:, :, bsl],
                            AF.Sigmoid,
                            scale=-1.0,
                        )
                elif j in (5, 6):
                    # DVE: recip, two quarters per offset
                    if "r1" not in st:
                        st["r1"] = ap_.tile([1, BLK], F32, tag="r1", name="r1")
                    for q in range(2):
                        qtr = (j - 5) * 2 + q
                        sl = slice(qtr * (BLK // 4), (qtr + 1) * (BLK // 4))
                        nc.vector.reciprocal_approx_fast(
                            out=st["r1"][:, sl], in_=st["sneg"][:, sl]
                        )
                elif j == 7:
                    e = st["e"] = ap_.tile([1, BLK], F32, tag="e", name="e")
                    nc.gpsimd.tensor_mul(e, st["spos"], st["r1"])
                elif j == 8:
                    ebc = st["ebc"] = ap_.tile(
                        [H, BLK], F32, tag="ebc", bufs=2, name="ebc"
                    )
                    nc.gpsimd.partition_broadcast(ebc, st["e"])
                elif j in (9, 10):
                    half = j - 9
                    if "g" not in st:
                        st["g"] = ap_.tile([H, BLK], F32, tag="g", bufs=2, name="g")
                    bsl = slice(half * (B // 2), (half + 1) * (B // 2))
                    hw = hist[:, 16 * k : 16 * k + 16, :].rearrange("p s b -> p b s")
                    gv = st["g"].rearrange("p (b s) -> p b s", s=16)
                    pev = st["ebc"].rearrange("p (b s) -> p b s", s=16)
                    nc.gpsimd.tensor_mul(
                        gv[:, bsl, :], hw[:, bsl, :], pev[:, bsl, :]
                    )

            def attn_tail(k, j):
                st = attn_state.setdefault(k, {})
                if j in (1, 2, 3, 4):
                    qtr = j - 1
                    if "red" not in st:
                        st["red"] = ap_.tile([H, B], F32, tag="red", name="red")
                    bsl = slice(qtr * (B // 4), (qtr + 1) * (B // 4))
                    gv = st["g"].rearrange("p (b s) -> p b s", s=16)
                    nc.vector.tensor_reduce(
                        st["red"][:, bsl],
                        gv[:, bsl, :],
                        axis=mybir.AxisListType.X,
                        op=ALU.add,
                    )
                elif j in (8, 9):
                    half = j - 8
                    if "ered" not in st:
                        st["ered"] = ap_.tile([1, B], F32, tag="ered", name="ered")
                    bsl = slice(half * (B // 2), (half + 1) * (B // 2))
                    ev = st["e"].rearrange("o (b s) -> o b s", s=16)
                    nc.vector.tensor_reduce(
                        st["ered"][:, bsl],
                        ev[:, bsl, :],
                        axis=mybir.AxisListType.X,
                        op=ALU.add,
                    )
                elif j == 10:
                    nc.tensor.matmul(
                        psacc[0:H],
                        lhsT=ident,
                        rhs=st["red"],
                        start=(k == 0),
                        stop=False,
                        skip_group_check=True,
                    )
                    nc.tensor.matmul(
                        psacc[H : H + 1],
                        lhsT=ones_f[:, 0:1],
                        rhs=st["ered"],
                        start=(k == 0),
                        stop=False,
                        skip_group_check=True,
                    )
                    attn_state.pop(k, None)

            # ---- phase 2: recurrence ----
            h2_prev = z2
            for t in range(S):
                k = t // 16
                w = t % 16
                cs = slice(w * B, (w + 1) * B)  # psRZ window
                wsl = slice(w * W2, (w + 1) * W2)  # ps2 interleaved window
                pr = psrz_cur[k % 2]
                p2 = ps2_cur[k % 2]
                hp_odd = h2_prev.rearrange("p (b two) -> p b two", two=2)[:, :, 1]

                nc.tensor.matmul(
                    pr[:, cs], lhsT=s1, rhs=hp_odd[0:H], start=False, stop=True,
                    skip_group_check=True,
                )
                p2v = p2[H : 2 * H, wsl].rearrange("p (b two) -> p b two", two=2)
                nc.tensor.matmul(
                    p2v[:, :, 0], lhsT=s2a, rhs=hp_odd, start=True, stop=True
                )
                nc.tensor.matmul(
                    p2[0:1, w * W2 : w * W2 + B],
                    lhsT=wattn,
                    rhs=hp_odd[0:H],
                    start=True,
                    stop=True,
                )

                F2 = f2_bufs[t % 2]
                f2v = F2.rearrange("p (b two) -> p b two", two=2)
                nc.scalar.activation(f2v[:, :, 1], pr[:, cs], AF.Sigmoid, bias=biasrzn)

                qp = sp.tile([2 * H, W2], F32, tag="qp", name="qp")
                nc.vector.tensor_tensor_scan(
                    qp[H : 2 * H],
                    F2[H : 2 * H],
                    p2[H : 2 * H, wsl],
                    0.0,
                    op0=ALU.mult,
                    op1=ALU.add,
                )

                w1 = sp.tile([H, B], F32, tag="w1", name="w1")
                nc.vector.tensor_mul(w1, f2v[0:H, :, 1], hp_odd[0:H])
                Y1 = sp.tile([H, W2], F32, tag="Y1", name="Y1")
                y1v = Y1.rearrange("p (b two) -> p b two", two=2)
                nc.vector.tensor_tensor(
                    y1v[:, :, 1], hp_odd[0:H], w1, op=ALU.subtract
                )

                nc.scalar.activation(
                    y1v[:, :, 0],
                    qp[H : 2 * H].rearrange("p (b two) -> p b two", two=2)[:, :, 1],
                    AF.Tanh,
                    bias=bin_[H : 2 * H],
                )

                h2 = h2_bufs[t % 2]
                nc.vector.tensor_tensor_scan(
                    h2[0:H], F2[0:H], Y1, 0.0, op0=ALU.mult, op1=ALU.add
                )
                h2_prev = h2

                # contiguous hist slot for attention (DVE, right after Y so the
                # h2 WAR stays a free same-engine dep)
                nc.vector.tensor_copy(
                    hist[:, t + 1, :],
                    h2[0:H].rearrange("p (b two) -> p b two", two=2)[:, :, 1],
                )

                # ---- interleaved block-level work (attention pipelined
                # across two blocks; demoted priority so the scheduler never
                # slots an extra ahead of a near-ready chain op) ----
                with tc.high_priority(offset=-1000000):
                    if k >= 1:
                        attn_head(k - 1, t % 16)
                    if k >= 2:
                        attn_tail(k - 2, t % 16)
                    if t % 16 == 4 and k + 1 < nblk:
                        pn_pend = phase1(k + 1)
                    elif t % 16 == 5 and k + 1 < nblk:
                        xn_flush(k + 1, pn_pend, 0)
                    elif t % 16 == 6 and k + 1 < nblk:
                        xn_flush(k + 1, pn_pend, 1)

            # ---- epilogue ----
            for j in range(1, 11):
                attn_tail(nblk - 2, j)
            for j in range(1, 11):
                attn_head(nblk - 1, j)
            for j in range(1, 11):
                attn_tail(nblk - 1, j)

            fin = ps2_tile(nblk)[0:1, 0:B]
            nc.tensor.matmul(fin, lhsT=wattn, rhs=hist[:, S, :], start=True, stop=True)
            sp512 = ap_.tile([1, B], F32, tag="sp512", name="sp512")
            nc.scalar.activation(sp512, fin, AF.Sigmoid)
            sn512 = ap_.tile([1, B], F32, tag="sn512", name="sn512")
            nc.scalar.activation(sn512, fin, AF.Sigmoid, scale=-1.0)
            r512 = ap_.tile([1, B], F32, tag="r512", name="r512")
            nc.vector.reciprocal_approx_fast(out=r512, in_=sn512)
            e512 = ap_.tile([1, B], F32, tag="e512", name="e512")
            nc.vector.tensor_mul(e512, sp512, r512)
            pe512 = ap_.tile([H, B], F32, tag="pe512", name="pe512")
            nc.gpsimd.partition_broadcast(pe512, e512)
            g512 = ap_.tile([H, B], F32, tag="g512", name="g512")
            nc.vector.tensor_mul(g512, hist[:, S, :], pe512)
            nc.tensor.matmul(
                psacc[0:H], lhsT=ident, rhs=g512, start=False, stop=True,
                skip_group_check=True,
            )
            nc.tensor.matmul(
                psacc[H : H + 1], lhsT=ones_f[:, 0:1], rhs=e512, start=False,
                stop=True, skip_group_check=True,
            )

            # normalize + FC
            acce = ap_.tile([1, B], F32, tag="acce", name="acce")
            nc.vector.tensor_scalar_add(acce, psacc[H : H + 1], -1.0)
            rinv = ap_.tile([1, B], F32, tag="rinv", name="rinv")
            rscr = ap_.tile([1, B], F32, tag="rscr", name="rscr")
            nc.vector.reciprocal_approx_accurate(out=rinv, in_=acce, scratch=rscr)
            rb = ap_.tile([H, B], F32, tag="rb", name="rb")
            nc.gpsimd.partition_broadcast(rb, rinv)
            ctxn = ap_.tile([H, B], F32, tag="ctxn", name="ctxn")
            nc.vector.tensor_mul(ctxn, psacc[0:H], rb)
            psy = ps.tile([2 * H, BLK], F32, tag="rz", bufs=2, name="psy")[0:C, 0:B]
            nc.tensor.matmul(psy, lhsT=sfc, rhs=ctxn, start=True, stop=True)
            y_sb = ap_.tile([C, B], F32, tag="ysb", name="y_sb")
            nc.vector.tensor_scalar_add(y_sb, psy, bfc)
            nc.sync.dma_start(out=y_d.ap().rearrange("b c -> c b"), in_=y_sb)

    nc.compile()
    return nc


def prep_core_inputs(x_shard, w_ih, w_hh, b_ih, b_hh, w_attn, w_fc, b_fc):
    """Build the per-core in_map from a [B, S, I] f32 shard + full params."""
    B, S, I_ = x_shard.shape
    xT = np.ascontiguousarray(
        x_shard.transpose(2, 1, 0).reshape(I_, B * S)
    ).astype(np.float32)
    # psRZ rows 0:64 = NEGATED z-preact; rows 64:128 = r-preact
    wihrz = np.concatenate([-w_ih[H : 2 * H].T, w_ih[0:H].T], axis=1)
    wihn = w_ih[2 * H : 3 * H].T
    s1 = np.concatenate([-w_hh[H : 2 * H].T, w_hh[0:H].T], axis=1)
    s2a = np.concatenate(
        [w_hh[2 * H : 3 * H].T, b_hh[2 * H : 3 * H].reshape(1, H)], axis=0
    )
    biasrzn = np.concatenate(
        [-(b_ih[H : 2 * H] + b_hh[H : 2 * H]), b_ih[0:H] + b_hh[0:H]]
    ).reshape(2 * H, 1)
    bin_ = np.zeros((2 * H, 1), np.float32)
    bin_[H : 2 * H, 0] = b_ih[2 * H : 3 * H]
    import ml_dtypes

    return {
        "xT": xT.astype(ml_dtypes.bfloat16),
        "wihrz": np.ascontiguousarray(wihrz).astype(ml_dtypes.bfloat16),
        "wihn": np.ascontiguousarray(wihn).astype(ml_dtypes.bfloat16),
        "s1": np.ascontiguousarray(s1, dtype=np.float32),
        "s2a": np.ascontiguousarray(s2a, dtype=np.float32),
        "wattn": np.ascontiguousarray(w_attn.T, dtype=np.float32),
        "biasrzn": np.ascontiguousarray(biasrzn, dtype=np.float32),
        "bin": bin_,
        "ones_f": np.ones((1, H), dtype=np.float32),
        "ident": np.eye(H, dtype=np.float32),
        "sfc": np.ascontiguousarray(w_fc.T, dtype=np.float32),
        "bfc": np.ascontiguousarray(b_fc.reshape(C, 1), dtype=np.float32),
    }


_NC_CACHE = {}


def kernel(x, w_ih, w_hh, b_ih, b_hh, w_attn, b_attn, w_fc, b_fc):
    x = np.asarray(x, dtype=np.float32)
    w_ih = np.asarray(w_ih, dtype=np.float32)
    w_hh = np.asarray(w_hh, dtype=np.float32)
    b_ih = np.asarray(b_ih, dtype=np.float32)
    b_hh = np.asarray(b_hh, dtype=np.float32)
    w_attn = np.asarray(w_attn, dtype=np.float32)
    w_fc = np.asarray(w_fc, dtype=np.float32)
    b_fc = np.asarray(b_fc, dtype=np.float32)

    Bfull, S, _ = x.shape
    B = Bfull // N_CORES
    key = (S, B)
    if key not in _NC_CACHE:
        _NC_CACHE[key] = build_program(S, B, num_devices=N_CORES)
    nc = _NC_CACHE[key]

    in_maps = []
    for ci in range(N_CORES):
        shard = x[ci * B : (ci + 1) * B]
        in_maps.append(
            prep_core_inputs(shard, w_ih, w_hh, b_ih, b_hh, w_attn, w_fc, b_fc)
        )
    res = bass_utils.run_bass_kernel_spmd(nc, in_maps, core_ids=list(range(N_CORES)))
    out = np.concatenate([res.results[ci]["y"] for ci in range(N_CORES)], axis=0)
    return out.astype(np.float32)


# revision 4
# speedup vs baseline: 1.2748x; 1.0039x over previous
"""AttentionGRU Trainium2 kernel v3: scan-fused recurrence, 8-core batch-parallel.

Per-step chain (5 links, was 7):
  mm1 (W_rz·h into psRZ, accumulating onto phase-1's xg) ->
  sigma (F2 odds = [u; r], z-preact negated so sigma gives u=1-z directly) ->
  X-scan: tensor_tensor_scan(F2[64:128]=[0|r], ps2win=[hn_b|xn]) pairs ->
     even: 0*state+hn_b ; odd: r*hn_b+xn = q ->
  tanh (q odds -> nt at Y1 evens, bias=b_in) ->
  Y-scan: scan(F2[0:64]=[0|u], Y1=[nt|m2]) -> odd: u*nt+m2 = h'
w1=u*h, m2=h-w1 fill the tanh window on DVE; a DVE copy moves h' odds into the
contiguous hist slot for the attention; 3 matmuls/step (W_rz, W_n_aug, w_attn).
b_hn rides a prefilled ones-row (h2 row 64 odds = 1, s2_aug row 64 = b_hn).
Attention (pipelined across two blocks, priority-demoted, chunked to fit the
chain's semaphore windows): e^l = sigma(l)*recip(sigma(-l)) with both sigmoids
in-table (no Exp table reloads) and a single custom-DVE reciprocal; Pool does
the e-multiply, partition-broadcast of e, and G = hist*E; DVE reduces G over t;
ctx and sum_e accumulate in one PSUM bank via identity-matmul accumulation held
open across all blocks (sum_e gets a -1 correction for the h_{-1}=0 slot).
Steady-state step = 1757 ns on the TimelineSim cost model.
"""

import sys

sys.path.insert(0, "/opt/trn_rl_repo")

import numpy as np

import concourse.bacc as bacc
import concourse.tile as tile
from concourse import mybir
from concourse import bass_utils

F32 = mybir.dt.float32
BF16 = mybir.dt.bfloat16
AF = mybir.ActivationFunctionType
ALU = mybir.AluOpType

H = 64
I = 128
C = 2
N_CORES = 8


def build_program(S: int, B: int = 32, num_devices: int = N_CORES):
    TOK = B * S
    nblk = S // 16
    BLK = 16 * B  # tokens per block
    W2 = 2 * B  # interleaved window width (64)
    nc = bacc.Bacc(
        "TRN2", target_bir_lowering=False, debug=False, num_devices=num_devices
    )

    xT_d = nc.dram_tensor("xT", [I, TOK], BF16, kind="ExternalInput")
    wihrz_d = nc.dram_tensor("wihrz", [I, 2 * H], BF16, kind="ExternalInput")
    wihn_d = nc.dram_tensor("wihn", [I, H], BF16, kind="ExternalInput")
    s1_d = nc.dram_tensor("s1", [H, 2 * H], F32, kind="ExternalInput")
    s2a_d = nc.dram_tensor("s2a", [H + 1, H], F32, kind="ExternalInput")
    wattn_d = nc.dram_tensor("wattn", [H, 1], F32, kind="ExternalInput")
    biasrzn_d = nc.dram_tensor("biasrzn", [2 * H, 1], F32, kind="ExternalInput")
    bin_d = nc.dram_tensor("bin", [2 * H, 1], F32, kind="ExternalInput")
    ones_d = nc.dram_tensor("ones_f", [1, H], F32, kind="ExternalInput")
    ident_d = nc.dram_tensor("ident", [H, H], F32, kind="ExternalInput")
    sfc_d = nc.dram_tensor("sfc", [H, C], F32, kind="ExternalInput")
    bfc_d = nc.dram_tensor("bfc", [C, 1], F32, kind="ExternalInput")
    y_d = nc.dram_tensor("y", [B, C], F32, kind="ExternalOutput")

    with tile.TileContext(nc) as tc:
        with (
            tc.tile_pool(name="const", bufs=1) as const,
            tc.tile_pool(name="big", bufs=1) as bigp,
            tc.tile_pool(name="step", bufs=2) as sp,
            tc.tile_pool(name="attn", bufs=2) as ap_,
            tc.tile_pool(name="ps", bufs=1, space="PSUM") as ps,
        ):
            # ---- recurrence-critical loads first (shortest path to step 0) ----
            xT = bigp.tile([I, TOK], BF16)
            n_ld = 8
            sl0 = slice(0, TOK // n_ld)
            nc.sync.dma_start(out=xT[:, sl0], in_=xT_d.ap()[:, sl0])
            wihrz = const.tile([I, 2 * H], BF16)
            nc.scalar.dma_start(out=wihrz, in_=wihrz_d.ap())
            wihn = const.tile([I, H], BF16)
            nc.scalar.dma_start(out=wihn, in_=wihn_d.ap())
            s1 = const.tile([H, 2 * H], F32)
            nc.scalar.dma_start(out=s1, in_=s1_d.ap())
            s2a = const.tile([H + 1, H], F32)
            nc.scalar.dma_start(out=s2a, in_=s2a_d.ap())
            wattn = const.tile([H, 1], F32)
            nc.scalar.dma_start(out=wattn, in_=wattn_d.ap())
            biasrzn = const.tile([2 * H, 1], F32)
            nc.scalar.dma_start(out=biasrzn, in_=biasrzn_d.ap())
            bin_ = const.tile([2 * H, 1], F32)
            nc.scalar.dma_start(out=bin_, in_=bin_d.ap())
            # attention/epilogue-only consts ride the SP queue behind xT
            ones_f = const.tile([1, H], F32)
            nc.sync.dma_start(out=ones_f, in_=ones_d.ap())
            ident = const.tile([H, H], F32)
            nc.sync.dma_start(out=ident, in_=ident_d.ap())
            sfc = const.tile([H, C], F32)
            nc.sync.dma_start(out=sfc, in_=sfc_d.ap())
            bfc = const.tile([C, 1], F32)
            nc.sync.dma_start(out=bfc, in_=bfc_d.ap())

            # ---- rest of x ----
            for cch in range(1, n_ld):
                sl = slice(cch * (TOK // n_ld), (cch + 1) * (TOK // n_ld))
                nc.sync.dma_start(out=xT[:, sl], in_=xT_d.ap()[:, sl])
            hist = bigp.tile([H, S + 1, B], F32)  # slot t = h_{t-1} (contiguous)
            nc.vector.memset(hist[:, 0, :], 0.0)

            # explicit ping-pong tiles with persistent prefill (evens=0 for F2,
            # ones row for h2) — plain tiles, not tag-rotated, so the prefill
            # survives across steps
            f2_bufs = []
            h2_bufs = []
            for i in range(2):
                f = bigp.tile([2 * H, W2], F32, name=f"F2buf{i}")
                nc.vector.memset(f, 0.0)
                f2_bufs.append(f)
                h = bigp.tile([H + 1, W2], F32, name=f"h2buf{i}")
                nc.vector.memset(h[H : H + 1], 1.0)
                h2_bufs.append(h)
            z2 = bigp.tile([H + 1, W2], F32, name="z2")
            nc.vector.memset(z2, 0.0)
            nc.vector.memset(z2[H : H + 1], 1.0)

            # ---- PSUM tiles ----
            def psrz_tile(k):
                return ps.tile([2 * H, BLK], F32, tag="rz", bufs=2, name=f"psrz{k}")

            def ps2_tile(k):
                # [hn|xn] interleaved windows (rows 64:128) + l strip (row 0)
                return ps.tile(
                    [2 * H, 2 * BLK], F32, tag="p2", bufs=2, name=f"ps2_{k}"
                )

            psacc = ps.tile([2 * H, BLK], F32, tag="acc", bufs=1, name="psacc")[
                :, 0:B
            ]

            psrz_cur = [None, None]
            ps2_cur = [None, None]

            def phase1(k):
                t0 = k * BLK
                pr = psrz_tile(k)
                psrz_cur[k % 2] = pr
                nc.tensor.matmul(
                    pr, lhsT=wihrz, rhs=xT[:, t0 : t0 + BLK], start=True, stop=True
                )
                p2 = ps2_tile(k)
                ps2_cur[k % 2] = p2
                pn = ps.tile([2 * H, BLK], F32, tag="pn", bufs=1, name=f"psn{k}")
                nc.tensor.matmul(
                    pn[H : 2 * H],
                    lhsT=wihn,
                    rhs=xT[:, t0 : t0 + BLK],
                    start=True,
                    stop=True,
                )
                return pn

            def xn_flush(k, pn, half):
                # xn -> odd columns of the interleaved ps2 windows (ACT write,
                # strided PSUM out is fine for the scalar engine)
                p2 = ps2_cur[k % 2]
                ssl = slice(half * 8, (half + 1) * 8)
                p2v = p2[H : 2 * H].rearrange(
                    "p (s b two) -> p s b two", b=B, two=2
                )
                pnv = pn[H : 2 * H].rearrange("p (s b) -> p s b", b=B)
                nc.scalar.activation(p2v[:, ssl, :, 1], pnv[:, ssl, :], AF.Identity)

            pn0 = phase1(0)
            xn_flush(0, pn0, 0)
            xn_flush(0, pn0, 1)

            # attention-block state
            attn_state = {}

            def attn_head(k, j):
                st = attn_state.setdefault(k, {})
                p2 = ps2_cur[k % 2]
                lview = p2[0:1].rearrange("o (s c) -> o s c", c=W2)[:, :, 0:B]
                if j in (1, 2):
                    # ACT: sigma(+l), b-quarters x2 per window
                    if "spos" not in st:
                        st["spos"] = ap_.tile([1, BLK], F32, tag="spos", name="spos")
                    for q in range(2):
                        qtr = (j - 1) * 2 + q
                        bsl = slice(qtr * (B // 4), (qtr + 1) * (B // 4))
                        nc.scalar.activation(
                            st["spos"].rearrange("o (b s) -> o s b", s=16)[:, :, bsl],
                            lview[:, :, bsl],
                            AF.Sigmoid,
                        )
                elif j in (3, 4):
                    if "sneg" not in st:
                        st["sneg"] = ap_.tile([1, BLK], F32, tag="sneg", name="sneg")
                    for q in range(2):
                        qtr = (j - 3) * 2 + q
                        bsl = slice(qtr * (B // 4), (qtr + 1) * (B // 4))
                        nc.scalar.activation(
                            st["sneg"].rearrange("o (b s) -> o s b", s=16)[:, :, bsl],
                            lview[:, :, bsl],
                            AF.Sigmoid,
                            scale=-1.0,
                        )
                elif j in (5, 6):
                    # DVE: recip, two quarters per offset
                    if "r1" not in st:
                        st["r1"] = ap_.tile([1, BLK], F32, tag="r1", name="r1")
                    for q in range(2):
                        qtr = (j - 5) * 2 + q
                        sl = slice(qtr * (BLK // 4), (qtr + 1) * (BLK // 4))
                        nc.vector.reciprocal_approx_fast(
                            out=st["r1"][:, sl], in_=st["sneg"][:, sl]
                        )
                elif j == 7:
                    e = st["e"] = ap_.tile([1, BLK], F32, tag="e", name="e")
                    nc.gpsimd.tensor_mul(e, st["spos"], st["r1"])
                elif j == 8:
                    ebc = st["ebc"] = ap_.tile(
                        [H, BLK], F32, tag="ebc", bufs=2, name="ebc"
                    )
                    nc.gpsimd.partition_broadcast(ebc, st["e"])
                elif j in (9, 10):
                    half = j - 9
                    if "g" not in st:
                        st["g"] = ap_.tile([H, BLK], F32, tag="g", bufs=2, name="g")
                    bsl = slice(half * (B // 2), (half + 1) * (B // 2))
                    hw = hist[:, 16 * k : 16 * k + 16, :].rearrange("p s b -> p b s")
                    gv = st["g"].rearrange("p (b s) -> p b s", s=16)
                    pev = st["ebc"].rearrange("p (b s) -> p b s", s=16)
                    nc.gpsimd.tensor_mul(
                        gv[:, bsl, :], hw[:, bsl, :], pev[:, bsl, :]
                    )

            def attn_tail(k, j):
                st = attn_state.setdefault(k, {})
                if j in (1, 2, 3, 4):
                    qtr = j - 1
                    if "red" not in st:
                        st["red"] = ap_.tile([H, B], F32, tag="red", name="red")
                    bsl = slice(qtr * (B // 4), (qtr + 1) * (B // 4))
                    gv = st["g"].rearrange("p (b s) -> p b s", s=16)
                    nc.vector.tensor_reduce(
                        st["red"][:, bsl],
                        gv[:, bsl, :],
                        axis=mybir.AxisListType.X,
                        op=ALU.add,
                    )
                elif j in (8, 9):
                    half = j - 8
                    if "ered" not in st:
                        st["ered"] = ap_.tile([1, B], F32, tag="ered", name="ered")
                    bsl = slice(half * (B // 2), (half + 1) * (B // 2))
                    ev = st["e"].rearrange("o (b s) -> o b s", s=16)
                    nc.vector.tensor_reduce(
                        st["ered"][:, bsl],
                        ev[:, bsl, :],
                        axis=mybir.AxisListType.X,
                        op=ALU.add,
                    )
                elif j == 10:
                    nc.tensor.matmul(
                        psacc[0:H],
                        lhsT=ident,
                        rhs=st["red"],
                        start=(k == 0),
                        stop=False,
                        skip_group_check=True,
                    )
                    nc.tensor.matmul(
                        psacc[H : H + 1],
                        lhsT=ones_f[:, 0:1],
                        rhs=st["ered"],
                        start=(k == 0),
                        stop=False,
                        skip_group_check=True,
                    )
                    attn_state.pop(k, None)

            # ---- phase 2: recurrence ----
            h2_prev = z2
            for t in range(S):
                k = t // 16
                w = t % 16
                cs = slice(w * B, (w + 1) * B)  # psRZ window
                wsl = slice(w * W2, (w + 1) * W2)  # ps2 interleaved window
                pr = psrz_cur[k % 2]
                p2 = ps2_cur[k % 2]
                hp_odd = h2_prev.rearrange("p (b two) -> p b two", two=2)[:, :, 1]

                nc.tensor.matmul(
                    pr[:, cs], lhsT=s1, rhs=hp_odd[0:H], start=False, stop=True,
                    skip_group_check=True,
                )
                p2v = p2[H : 2 * H, wsl].rearrange("p (b two) -> p b two", two=2)
                nc.tensor.matmul(
                    p2v[:, :, 0], lhsT=s2a, rhs=hp_odd, start=True, stop=True
                )
                nc.tensor.matmul(
                    p2[0:1, w * W2 : w * W2 + B],
                    lhsT=wattn,
                    rhs=hp_odd[0:H],
                    start=True,
                    stop=True,
                )

                F2 = f2_bufs[t % 2]
                f2v = F2.rearrange("p (b two) -> p b two", two=2)
                nc.scalar.activation(f2v[:, :, 1], pr[:, cs], AF.Sigmoid, bias=biasrzn)

                qp = sp.tile([2 * H, W2], F32, tag="qp", name="qp")
                nc.vector.tensor_tensor_scan(
                    qp[H : 2 * H],
                    F2[H : 2 * H],
                    p2[H : 2 * H, wsl],
                    0.0,
                    op0=ALU.mult,
                    op1=ALU.add,
                )

                w1 = sp.tile([H, B], F32, tag="w1", name="w1")
                nc.vector.tensor_mul(w1, f2v[0:H, :, 1], hp_odd[0:H])
                Y1 = sp.tile([H, W2], F32, tag="Y1", name="Y1")
                y1v = Y1.rearrange("p (b two) -> p b two", two=2)
                nc.vector.tensor_tensor(
                    y1v[:, :, 1], hp_odd[0:H], w1, op=ALU.subtract
                )

                nc.scalar.activation(
                    y1v[:, :, 0],
                    qp[H : 2 * H].rearrange("p (b two) -> p b two", two=2)[:, :, 1],
                    AF.Tanh,
                    bias=bin_[H : 2 * H],
                )

                h2 = h2_bufs[t % 2]
                nc.vector.tensor_tensor_scan(
                    h2[0:H], F2[0:H], Y1, 0.0, op0=ALU.mult, op1=ALU.add
                )
                h2_prev = h2

                # contiguous hist slot for attention (DVE, right after Y so the
                # h2 WAR stays a free same-engine dep)
                nc.vector.tensor_copy(
                    hist[:, t + 1, :],
                    h2[0:H].rearrange("p (b two) -> p b two", two=2)[:, :, 1],
                )

                # ---- interleaved block-level work (attention pipelined
                # across two blocks; demoted priority so the scheduler never
                # slots an extra ahead of a near-ready chain op) ----
                with tc.high_priority(offset=-1000000):
                    if k >= 1:
                        attn_head(k - 1, t % 16)
                    if k >= 2:
                        attn_tail(k - 2, t % 16)
                    if t % 16 == 4 and k + 1 < nblk:
                        pn_pend = phase1(k + 1)
                    elif t % 16 == 5 and k + 1 < nblk:
                        xn_flush(k + 1, pn_pend, 0)
                    elif t % 16 == 6 and k + 1 < nblk:
                        xn_flush(k + 1, pn_pend, 1)

            # ---- epilogue ----
            for j in range(1, 11):
                attn_tail(nblk - 2, j)
            for j in range(1, 11):
                attn_head(nblk - 1, j)
            for j in range(1, 11):
                attn_tail(nblk - 1, j)

            fin = ps2_tile(nblk)[0:1, 0:B]
            nc.tensor.matmul(fin, lhsT=wattn, rhs=hist[:, S, :], start=True, stop=True)
            sp512 = ap_.tile([1, B], F32, tag="sp512", name="sp512")
            nc.scalar.activation(sp512, fin, AF.Sigmoid)
            sn512 = ap_.tile([1, B], F32, tag="sn512", name="sn512")
            nc.scalar.activation(sn512, fin, AF.Sigmoid, scale=-1.0)
            r512 = ap_.tile([1, B], F32, tag="r512", name="r512")
            nc.vector.reciprocal_approx_fast(out=r512, in_=sn512)
            e512 = ap_.tile([1, B], F32, tag="e512", name="e512")
            nc.vector.tensor_mul(e512, sp512, r512)
            pe512 = ap_.tile([H, B], F32, tag="pe512", name="pe512")
            nc.gpsimd.partition_broadcast(pe512, e512)
            g512 = ap_.tile([H, B], F32, tag="g512", name="g512")
            nc.vector.tensor_mul(g512, hist[:, S, :], pe512)
            nc.tensor.matmul(
                psacc[0:H], lhsT=ident, rhs=g512, start=False, stop=True,
                skip_group_check=True,
            )
            nc.tensor.matmul(
                psacc[H : H + 1], lhsT=ones_f[:, 0:1], rhs=e512, start=False,
                stop=True, skip_group_check=True,
            )

            # normalize + FC
            acce = ap_.tile([1, B], F32, tag="acce", name="acce")
            nc.vector.tensor_scalar_add(acce, psacc[H : H + 1], -1.0)
            rinv = ap_.tile([1, B], F32, tag="rinv", name="rinv")
            rscr = ap_.tile([1, B], F32, tag="rscr", name="rscr")
            nc.vector.reciprocal_approx_accurate(out=rinv, in_=acce, scratch=rscr)
            rb = ap_.tile([H, B], F32, tag="rb", name="rb")
            nc.gpsimd.partition_broadcast(rb, rinv)
            ctxn = ap_.tile([H, B], F32, tag="ctxn", name="ctxn")
            nc.vector.tensor_mul(ctxn, psacc[0:H], rb)
            psy = ps.tile([2 * H, BLK], F32, tag="rz", bufs=2, name="psy")[0:C, 0:B]
            nc.tensor.matmul(psy, lhsT=sfc, rhs=ctxn, start=True, stop=True)
            y_sb = ap_.tile([C, B], F32, tag="ysb", name="y_sb")
            nc.vector.tensor_scalar_add(y_sb, psy, bfc)
            nc.sync.dma_start(out=y_d.ap().rearrange("b c -> c b"), in_=y_sb)

    nc.compile()
    return nc


def prep_core_inputs(x_shard, w_ih, w_hh, b_ih, b_hh, w_attn, w_fc, b_fc):
    """Build the per-core in_map from a [B, S, I] f32 shard + full params."""
    B, S, I_ = x_shard.shape
    xT = np.ascontiguousarray(
        x_shard.transpose(2, 1, 0).reshape(I_, B * S)
    ).astype(np.float32)
    # psRZ rows 0:64 = NEGATED z-preact; rows 64:128 = r-preact
    wihrz = np.concatenate([-w_ih[H : 2 * H].T, w_ih[0:H].T], axis=1)
    wihn = w_ih[2 * H : 3 * H].T
    s1 = np.concatenate([-w_hh[H : 2 * H].T, w_hh[0:H].T], axis=1)
    s2a = np.concatenate(
        [w_hh[2 * H : 3 * H].T, b_hh[2 * H : 3 * H].reshape(1, H)], axis=0
    )
    biasrzn = np.concatenate(
        [-(b_ih[H : 2 * H] + b_hh[H : 2 * H]), b_ih[0:H] + b_hh[0:H]]
    ).reshape(2 * H, 1)
    bin_ = np.zeros((2 * H, 1), np.float32)
    bin_[H : 2 * H, 0] = b_ih[2 * H : 3 * H]
    import ml_dtypes

    return {
        "xT": xT.astype(ml_dtypes.bfloat16),
        "wihrz": np.ascontiguousarray(wihrz).astype(ml_dtypes.bfloat16),
        "wihn": np.ascontiguousarray(wihn).astype(ml_dtypes.bfloat16),
        "s1": np.ascontiguousarray(s1, dtype=np.float32),
        "s2a": np.ascontiguousarray(s2a, dtype=np.float32),
        "wattn": np.ascontiguousarray(w_attn.T, dtype=np.float32),
        "biasrzn": np.ascontiguousarray(biasrzn, dtype=np.float32),
        "bin": bin_,
        "ones_f": np.ones((1, H), dtype=np.float32),
        "ident": np.eye(H, dtype=np.float32),
        "sfc": np.ascontiguousarray(w_fc.T, dtype=np.float32),
        "bfc": np.ascontiguousarray(b_fc.reshape(C, 1), dtype=np.float32),
    }


_NC_CACHE = {}


def kernel(x, w_ih, w_hh, b_ih, b_hh, w_attn, b_attn, w_fc, b_fc):
    x = np.asarray(x, dtype=np.float32)
    w_ih = np.asarray(w_ih, dtype=np.float32)
    w_hh = np.asarray(w_hh, dtype=np.float32)
    b_ih = np.asarray(b_ih, dtype=np.float32)
    b_hh = np.asarray(b_hh, dtype=np.float32)
    w_attn = np.asarray(w_attn, dtype=np.float32)
    w_fc = np.asarray(w_fc, dtype=np.float32)
    b_fc = np.asarray(b_fc, dtype=np.float32)

    Bfull, S, _ = x.shape
    B = Bfull // N_CORES
    key = (S, B)
    if key not in _NC_CACHE:
        _NC_CACHE[key] = build_program(S, B, num_devices=N_CORES)
    nc = _NC_CACHE[key]

    in_maps = []
    for ci in range(N_CORES):
        shard = x[ci * B : (ci + 1) * B]
        in_maps.append(
            prep_core_inputs(shard, w_ih, w_hh, b_ih, b_hh, w_attn, w_fc, b_fc)
        )
    res = bass_utils.run_bass_kernel_spmd(nc, in_maps, core_ids=list(range(N_CORES)))
    out = np.concatenate([res.results[ci]["y"] for ci in range(N_CORES)], axis=0)
    return out.astype(np.float32)


# revision 5
# speedup vs baseline: 1.3001x; 1.0199x over previous
"""AttentionGRU Trainium2 kernel v3: scan-fused recurrence, 8-core batch-parallel.

Per-step chain (5 links, was 7):
  mm1 (W_rz·h into psRZ, accumulating onto phase-1's xg) ->
  sigma (F2 odds = [u; r], z-preact negated so sigma gives u=1-z directly) ->
  X-scan: tensor_tensor_scan(F2[64:128]=[0|r], ps2win=[hn_b|xn]) pairs ->
     even: 0*state+hn_b ; odd: r*hn_b+xn = q ->
  tanh (q odds -> nt at Y1 evens, bias=b_in) ->
  Y-scan: scan(F2[0:64]=[0|u], Y1=[nt|m2]) -> odd: u*nt+m2 = h'
w1=u*h, m2=h-w1 fill the tanh window on DVE; a DVE copy moves h' odds into the
contiguous hist slot for the attention; 3 matmuls/step (W_rz, W_n_aug, w_attn).
b_hn rides a prefilled ones-row (h2 row 64 odds = 1, s2_aug row 64 = b_hn).
Attention (pipelined across two blocks, priority-demoted, chunked to fit the
chain's semaphore windows): e^l = sigma(l)*recip(sigma(-l)) with both sigmoids
in-table (no Exp table reloads) and a single custom-DVE reciprocal; Pool does
the e-multiply, partition-broadcast of e, and G = hist*E; DVE reduces G over t;
ctx and sum_e accumulate in one PSUM bank via identity-matmul accumulation held
open across all blocks (sum_e gets a -1 correction for the h_{-1}=0 slot).
Steady-state step = 1757 ns on the TimelineSim cost model.
"""

import sys

sys.path.insert(0, "/opt/trn_rl_repo")

import numpy as np

import concourse.bacc as bacc
from concourse.bass import _add_dep_helper
import concourse.tile as tile
from concourse import mybir
from concourse import bass_utils

F32 = mybir.dt.float32
BF16 = mybir.dt.bfloat16
AF = mybir.ActivationFunctionType
ALU = mybir.AluOpType

H = 64
I = 128
C = 2
N_CORES = 8


def build_program(S: int, B: int = 32, num_devices: int = N_CORES):
    TOK = B * S
    nblk = S // 16
    BLK = 16 * B  # tokens per block
    W2 = 2 * B  # interleaved window width (64)
    nc = bacc.Bacc(
        "TRN2", target_bir_lowering=False, debug=False, num_devices=num_devices
    )

    xT_d = nc.dram_tensor("xT", [I, TOK], BF16, kind="ExternalInput")
    wihrz_d = nc.dram_tensor("wihrz", [I, 2 * H], BF16, kind="ExternalInput")
    wihn_d = nc.dram_tensor("wihn", [I, H], BF16, kind="ExternalInput")
    s1_d = nc.dram_tensor("s1", [H, 2 * H], F32, kind="ExternalInput")
    s2a_d = nc.dram_tensor("s2a", [H + 1, H], F32, kind="ExternalInput")
    wattn_d = nc.dram_tensor("wattn", [H, 1], F32, kind="ExternalInput")
    biasrzn_d = nc.dram_tensor("biasrzn", [2 * H, 1], F32, kind="ExternalInput")
    bin_d = nc.dram_tensor("bin", [2 * H, 1], F32, kind="ExternalInput")
    ones_d = nc.dram_tensor("ones_f", [1, H], F32, kind="ExternalInput")
    ident_d = nc.dram_tensor("ident", [H, H], F32, kind="ExternalInput")
    sfc_d = nc.dram_tensor("sfc", [H, C], F32, kind="ExternalInput")
    bfc_d = nc.dram_tensor("bfc", [C, 1], F32, kind="ExternalInput")
    y_d = nc.dram_tensor("y", [B, C], F32, kind="ExternalOutput")

    with tile.TileContext(nc) as tc:
        with (
            tc.tile_pool(name="const", bufs=1) as const,
            tc.tile_pool(name="big", bufs=1) as bigp,
            tc.tile_pool(name="step", bufs=2) as sp,
            tc.tile_pool(name="attn", bufs=2) as ap_,
            tc.tile_pool(name="ps", bufs=1, space="PSUM") as ps,
        ):
            # ---- recurrence-critical loads first (shortest path to step 0) ----
            xT = bigp.tile([I, TOK], BF16)
            n_ld = 8
            sl0 = slice(0, TOK // n_ld)
            nc.sync.dma_start(out=xT[:, sl0], in_=xT_d.ap()[:, sl0])
            wihrz = const.tile([I, 2 * H], BF16)
            nc.scalar.dma_start(out=wihrz, in_=wihrz_d.ap())
            wihn = const.tile([I, H], BF16)
            nc.scalar.dma_start(out=wihn, in_=wihn_d.ap())
            s1 = const.tile([H, 2 * H], F32)
            nc.scalar.dma_start(out=s1, in_=s1_d.ap())
            s2a = const.tile([H + 1, H], F32)
            nc.scalar.dma_start(out=s2a, in_=s2a_d.ap())
            wattn = const.tile([H, 1], F32)
            nc.scalar.dma_start(out=wattn, in_=wattn_d.ap())
            biasrzn = const.tile([2 * H, 1], F32)
            nc.scalar.dma_start(out=biasrzn, in_=biasrzn_d.ap())
            bin_ = const.tile([2 * H, 1], F32)
            nc.scalar.dma_start(out=bin_, in_=bin_d.ap())
            # attention/epilogue-only consts ride the SP queue behind xT
            ones_f = const.tile([1, H], F32)
            nc.sync.dma_start(out=ones_f, in_=ones_d.ap())
            ident = const.tile([H, H], F32)
            nc.sync.dma_start(out=ident, in_=ident_d.ap())
            sfc = const.tile([H, C], F32)
            nc.sync.dma_start(out=sfc, in_=sfc_d.ap())
            bfc = const.tile([C, 1], F32)
            nc.sync.dma_start(out=bfc, in_=bfc_d.ap())

            # ---- rest of x ----
            for cch in range(1, n_ld):
                sl = slice(cch * (TOK // n_ld), (cch + 1) * (TOK // n_ld))
                nc.sync.dma_start(out=xT[:, sl], in_=xT_d.ap()[:, sl])
            hist = bigp.tile([H, S + 1, B], F32)  # slot t = h_{t-1} (contiguous)
            nc.vector.memset(hist[:, 0, :], 0.0)

            # explicit ping-pong tiles with persistent prefill (evens=0 for F2,
            # ones row for h2) — plain tiles, not tag-rotated, so the prefill
            # survives across steps
            f2_bufs = []
            h2_bufs = []
            for i in range(2):
                f = bigp.tile([2 * H, W2], F32, name=f"F2buf{i}")
                nc.vector.memset(f, 0.0)
                f2_bufs.append(f)
                h = bigp.tile([H + 1, W2], F32, name=f"h2buf{i}")
                nc.vector.memset(h[H : H + 1], 1.0)
                h2_bufs.append(h)
            z2 = bigp.tile([H + 1, W2], F32, name="z2")
            nc.vector.memset(z2, 0.0)
            nc.vector.memset(z2[H : H + 1], 1.0)

            # ---- PSUM tiles ----
            def psrz_tile(k):
                return ps.tile([2 * H, BLK], F32, tag="rz", bufs=2, name=f"psrz{k}")

            def ps2_tile(k):
                # [hn|xn] interleaved windows (rows 64:128) + l strip (row 0)
                return ps.tile(
                    [2 * H, 2 * BLK], F32, tag="p2", bufs=2, name=f"ps2_{k}"
                )

            psacc = ps.tile([2 * H, BLK], F32, tag="acc", bufs=1, name="psacc")[
                :, 0:B
            ]

            psrz_cur = [None, None]
            ps2_cur = [None, None]

            def phase1(k):
                t0 = k * BLK
                pr = psrz_tile(k)
                psrz_cur[k % 2] = pr
                nc.tensor.matmul(
                    pr, lhsT=wihrz, rhs=xT[:, t0 : t0 + BLK], start=True, stop=True
                )
                p2 = ps2_tile(k)
                ps2_cur[k % 2] = p2
                pn = ps.tile([2 * H, BLK], F32, tag="pn", bufs=1, name=f"psn{k}")
                nc.tensor.matmul(
                    pn[H : 2 * H],
                    lhsT=wihn,
                    rhs=xT[:, t0 : t0 + BLK],
                    start=True,
                    stop=True,
                )
                return pn

            def xn_flush(k, pn, half):
                # xn -> odd columns of the interleaved ps2 windows (ACT write,
                # strided PSUM out is fine for the scalar engine)
                p2 = ps2_cur[k % 2]
                ssl = slice(half * 8, (half + 1) * 8)
                p2v = p2[H : 2 * H].rearrange(
                    "p (s b two) -> p s b two", b=B, two=2
                )
                pnv = pn[H : 2 * H].rearrange("p (s b) -> p s b", b=B)
                nc.scalar.activation(p2v[:, ssl, :, 1], pnv[:, ssl, :], AF.Identity)

            pn0 = phase1(0)
            xn_flush(0, pn0, 0)
            xn_flush(0, pn0, 1)

            # attention-block state
            attn_state = {}

            def attn_head(k, j):
                st = attn_state.setdefault(k, {})
                p2 = ps2_cur[k % 2]
                lview = p2[0:1].rearrange("o (s c) -> o s c", c=W2)[:, :, 0:B]
                if j in (1, 2):
                    # ACT: sigma(+l), b-quarters x2 per window
                    if "spos" not in st:
                        st["spos"] = ap_.tile([1, BLK], F32, tag="spos", name="spos")
                    for q in range(2):
                        qtr = (j - 1) * 2 + q
                        bsl = slice(qtr * (B // 4), (qtr + 1) * (B // 4))
                        nc.scalar.activation(
                            st["spos"].rearrange("o (b s) -> o s b", s=16)[:, :, bsl],
                            lview[:, :, bsl],
                            AF.Sigmoid,
                        )
                elif j in (3, 4):
                    if "sneg" not in st:
                        st["sneg"] = ap_.tile([1, BLK], F32, tag="sneg", name="sneg")
                    for q in range(2):
                        qtr = (j - 3) * 2 + q
                        bsl = slice(qtr * (B // 4), (qtr + 1) * (B // 4))
                        nc.scalar.activation(
                            st["sneg"].rearrange("o (b s) -> o s b", s=16)[:, :, bsl],
                            lview[:, :, bsl],
                            AF.Sigmoid,
                            scale=-1.0,
                        )
                elif j in (5, 6):
                    # DVE: recip, two quarters per offset
                    if "r1" not in st:
                        st["r1"] = ap_.tile([1, BLK], F32, tag="r1", name="r1")
                    for q in range(2):
                        qtr = (j - 5) * 2 + q
                        sl = slice(qtr * (BLK // 4), (qtr + 1) * (BLK // 4))
                        nc.vector.reciprocal_approx_fast(
                            out=st["r1"][:, sl], in_=st["sneg"][:, sl]
                        )
                elif j == 7:
                    e = st["e"] = ap_.tile([1, BLK], F32, tag="e", name="e")
                    nc.gpsimd.tensor_mul(e, st["spos"], st["r1"])
                elif j == 8:
                    ebc = st["ebc"] = ap_.tile(
                        [H, BLK], F32, tag="ebc", bufs=2, name="ebc"
                    )
                    nc.gpsimd.partition_broadcast(ebc, st["e"])
                elif j in (9, 10):
                    half = j - 9
                    if "g" not in st:
                        st["g"] = ap_.tile([H, BLK], F32, tag="g", bufs=2, name="g")
                    bsl = slice(half * (B // 2), (half + 1) * (B // 2))
                    hw = hist[:, 16 * k : 16 * k + 16, :].rearrange("p s b -> p b s")
                    gv = st["g"].rearrange("p (b s) -> p b s", s=16)
                    pev = st["ebc"].rearrange("p (b s) -> p b s", s=16)
                    nc.gpsimd.tensor_mul(
                        gv[:, bsl, :], hw[:, bsl, :], pev[:, bsl, :]
                    )

            def attn_tail(k, j):
                st = attn_state.setdefault(k, {})
                if j in (1, 2, 3, 4):
                    qtr = j - 1
                    if "red" not in st:
                        st["red"] = ap_.tile([H, B], F32, tag="red", name="red")
                    bsl = slice(qtr * (B // 4), (qtr + 1) * (B // 4))
                    gv = st["g"].rearrange("p (b s) -> p b s", s=16)
                    nc.vector.tensor_reduce(
                        st["red"][:, bsl],
                        gv[:, bsl, :],
                        axis=mybir.AxisListType.X,
                        op=ALU.add,
                    )
                elif j in (8, 9):
                    half = j - 8
                    if "ered" not in st:
                        st["ered"] = ap_.tile([1, B], F32, tag="ered", name="ered")
                    bsl = slice(half * (B // 2), (half + 1) * (B // 2))
                    ev = st["e"].rearrange("o (b s) -> o b s", s=16)
                    nc.vector.tensor_reduce(
                        st["ered"][:, bsl],
                        ev[:, bsl, :],
                        axis=mybir.AxisListType.X,
                        op=ALU.add,
                    )
                elif j == 10:
                    nc.tensor.matmul(
                        psacc[0:H],
                        lhsT=ident,
                        rhs=st["red"],
                        start=(k == 0),
                        stop=False,
                        skip_group_check=True,
                    )
                    nc.tensor.matmul(
                        psacc[H : H + 1],
                        lhsT=ones_f[:, 0:1],
                        rhs=st["ered"],
                        start=(k == 0),
                        stop=False,
                        skip_group_check=True,
                    )
                    attn_state.pop(k, None)

            # ---- phase 2: recurrence ----
            h2_prev = z2
            for t in range(S):
                k = t // 16
                w = t % 16
                cs = slice(w * B, (w + 1) * B)  # psRZ window
                wsl = slice(w * W2, (w + 1) * W2)  # ps2 interleaved window
                pr = psrz_cur[k % 2]
                p2 = ps2_cur[k % 2]
                hp_odd = h2_prev.rearrange("p (b two) -> p b two", two=2)[:, :, 1]

                nc.tensor.matmul(
                    pr[:, cs], lhsT=s1, rhs=hp_odd[0:H], start=False, stop=True,
                    skip_group_check=True,
                )
                p2v = p2[H : 2 * H, wsl].rearrange("p (b two) -> p b two", two=2)
                nc.tensor.matmul(
                    p2v[:, :, 0], lhsT=s2a, rhs=hp_odd, start=True, stop=True
                )
                nc.tensor.matmul(
                    p2[0:1, w * W2 : w * W2 + B],
                    lhsT=wattn,
                    rhs=hp_odd[0:H],
                    start=True,
                    stop=True,
                )

                # copy the [hn|xn] window to SBUF in the DVE's idle window:
                # the X-scan then runs all-SBUF (-65ns exec, -65ns ack)
                hx = sp.tile([2 * H, W2], F32, tag="hx", name="hx")
                nc.vector.tensor_copy(hx[H : 2 * H], p2[H : 2 * H, wsl])

                F2 = f2_bufs[t % 2]
                f2v = F2.rearrange("p (b two) -> p b two", two=2)
                nc.scalar.activation(f2v[:, :, 1], pr[:, cs], AF.Sigmoid, bias=biasrzn)

                qp = sp.tile([2 * H, W2], F32, tag="qp", name="qp")
                i_x = nc.vector.tensor_tensor_scan(
                    qp[H : 2 * H],
                    F2[H : 2 * H],
                    hx[H : 2 * H],
                    0.0,
                    op0=ALU.mult,
                    op1=ALU.add,
                )

                # w1/m2 only matter at tanh-end; order them AFTER the X-scan
                # (sync-free edge) so they fill the tanh window instead of
                # delaying X at the sigma-release tie
                w1 = sp.tile([H, B], F32, tag="w1", name="w1")
                i_w1 = nc.vector.tensor_mul(w1, f2v[0:H, :, 1], hp_odd[0:H])
                _add_dep_helper(i_w1.ins, i_x.ins, sync=False,
                                reason="order w1 after X")
                Y1 = sp.tile([H, W2], F32, tag="Y1", name="Y1")
                y1v = Y1.rearrange("p (b two) -> p b two", two=2)
                nc.vector.tensor_tensor(
                    y1v[:, :, 1], hp_odd[0:H], w1, op=ALU.subtract
                )

                nc.scalar.activation(
                    y1v[:, :, 0],
                    qp[H : 2 * H].rearrange("p (b two) -> p b two", two=2)[:, :, 1],
                    AF.Tanh,
                    bias=bin_[H : 2 * H],
                )

                h2 = h2_bufs[t % 2]
                nc.vector.tensor_tensor_scan(
                    h2[0:H], F2[0:H], Y1, 0.0, op0=ALU.mult, op1=ALU.add
                )
                h2_prev = h2

                # contiguous hist slot for attention (DVE, right after Y so the
                # h2 WAR stays a free same-engine dep; demoted, consumed a
                # block later)
                with tc.high_priority(offset=-1000000):
                    nc.vector.tensor_copy(
                        hist[:, t + 1, :],
                        h2[0:H].rearrange("p (b two) -> p b two", two=2)[:, :, 1],
                    )

                # ---- interleaved block-level work (attention pipelined
                # across two blocks; demoted priority so the scheduler never
                # slots an extra ahead of a near-ready chain op) ----
                with tc.high_priority(offset=-1000000):
                    if k >= 1:
                        attn_head(k - 1, t % 16)
                    if k >= 2:
                        attn_tail(k - 2, t % 16)
                    if t % 16 == 4 and k + 1 < nblk:
                        pn_pend = phase1(k + 1)
                    elif t % 16 == 5 and k + 1 < nblk:
                        xn_flush(k + 1, pn_pend, 0)
                    elif t % 16 == 6 and k + 1 < nblk:
                        xn_flush(k + 1, pn_pend, 1)

            # ---- epilogue ----
            for j in range(1, 11):
                attn_tail(nblk - 2, j)
            for j in range(1, 11):
                attn_head(nblk - 1, j)
            for j in range(1, 11):
                attn_tail(nblk - 1, j)

            fin = ps2_tile(nblk)[0:1, 0:B]
            nc.tensor.matmul(fin, lhsT=wattn, rhs=hist[:, S, :], start=True, stop=True)
            sp512 = ap_.tile([1, B], F32, tag="sp512", name="sp512")
            nc.scalar.activation(sp512, fin, AF.Sigmoid)
            sn512 = ap_.tile([1, B], F32, tag="sn512", name="sn512")
            nc.scalar.activation(sn512, fin, AF.Sigmoid, scale=-1.0)
            r512 = ap_.tile([1, B], F32, tag="r512", name="r512")
            nc.vector.reciprocal_approx_fast(out=r512, in_=sn512)
            e512 = ap_.tile([1, B], F32, tag="e512", name="e512")
            nc.vector.tensor_mul(e512, sp512, r512)
            pe512 = ap_.tile([H, B], F32, tag="pe512", name="pe512")
            nc.gpsimd.partition_broadcast(pe512, e512)
            g512 = ap_.tile([H, B], F32, tag="g512", name="g512")
            nc.vector.tensor_mul(g512, hist[:, S, :], pe512)
            nc.tensor.matmul(
                psacc[0:H], lhsT=ident, rhs=g512, start=False, stop=True,
                skip_group_check=True,
            )
            nc.tensor.matmul(
                psacc[H : H + 1], lhsT=ones_f[:, 0:1], rhs=e512, start=False,
                stop=True, skip_group_check=True,
            )

            # normalize + FC
            acce = ap_.tile([1, B], F32, tag="acce", name="acce")
            nc.vector.tensor_scalar_add(acce, psacc[H : H + 1], -1.0)
            rinv = ap_.tile([1, B], F32, tag="rinv", name="rinv")
            rscr = ap_.tile([1, B], F32, tag="rscr", name="rscr")
            nc.vector.reciprocal_approx_accurate(out=rinv, in_=acce, scratch=rscr)
            rb = ap_.tile([H, B], F32, tag="rb", name="rb")
            nc.gpsimd.partition_broadcast(rb, rinv)
            ctxn = ap_.tile([H, B], F32, tag="ctxn", name="ctxn")
            nc.vector.tensor_mul(ctxn, psacc[0:H], rb)
            psy = ps.tile([2 * H, BLK], F32, tag="rz", bufs=2, name="psy")[0:C, 0:B]
            nc.tensor.matmul(psy, lhsT=sfc, rhs=ctxn, start=True, stop=True)
            y_sb = ap_.tile([C, B], F32, tag="ysb", name="y_sb")
            nc.vector.tensor_scalar_add(y_sb, psy, bfc)
            nc.sync.dma_start(out=y_d.ap().rearrange("b c -> c b"), in_=y_sb)

    nc.compile()
    return nc


def prep_core_inputs(x_shard, w_ih, w_hh, b_ih, b_hh, w_attn, w_fc, b_fc):
    """Build the per-core in_map from a [B, S, I] f32 shard + full params."""
    B, S, I_ = x_shard.shape
    xT = np.ascontiguousarray(
        x_shard.transpose(2, 1, 0).reshape(I_, B * S)
    ).astype(np.float32)
    # psRZ rows 0:64 = NEGATED z-preact; rows 64:128 = r-preact
    wihrz = np.concatenate([-w_ih[H : 2 * H].T, w_ih[0:H].T], axis=1)
    wihn = w_ih[2 * H : 3 * H].T
    s1 = np.concatenate([-w_hh[H : 2 * H].T, w_hh[0:H].T], axis=1)
    s2a = np.concatenate(
        [w_hh[2 * H : 3 * H].T, b_hh[2 * H : 3 * H].reshape(1, H)], axis=0
    )
    biasrzn = np.concatenate(
        [-(b_ih[H : 2 * H] + b_hh[H : 2 * H]), b_ih[0:H] + b_hh[0:H]]
    ).reshape(2 * H, 1)
    bin_ = np.zeros((2 * H, 1), np.float32)
    bin_[H : 2 * H, 0] = b_ih[2 * H : 3 * H]
    import ml_dtypes

    return {
        "xT": xT.astype(ml_dtypes.bfloat16),
        "wihrz": np.ascontiguousarray(wihrz).astype(ml_dtypes.bfloat16),
        "wihn": np.ascontiguousarray(wihn).astype(ml_dtypes.bfloat16),
        "s1": np.ascontiguousarray(s1, dtype=np.float32),
        "s2a": np.ascontiguousarray(s2a, dtype=np.float32),
        "wattn": np.ascontiguousarray(w_attn.T, dtype=np.float32),
        "biasrzn": np.ascontiguousarray(biasrzn, dtype=np.float32),
        "bin": bin_,
        "ones_f": np.ones((1, H), dtype=np.float32),
        "ident": np.eye(H, dtype=np.float32),
        "sfc": np.ascontiguousarray(w_fc.T, dtype=np.float32),
        "bfc": np.ascontiguousarray(b_fc.reshape(C, 1), dtype=np.float32),
    }


_NC_CACHE = {}


def kernel(x, w_ih, w_hh, b_ih, b_hh, w_attn, b_attn, w_fc, b_fc):
    x = np.asarray(x, dtype=np.float32)
    w_ih = np.asarray(w_ih, dtype=np.float32)
    w_hh = np.asarray(w_hh, dtype=np.float32)
    b_ih = np.asarray(b_ih, dtype=np.float32)
    b_hh = np.asarray(b_hh, dtype=np.float32)
    w_attn = np.asarray(w_attn, dtype=np.float32)
    w_fc = np.asarray(w_fc, dtype=np.float32)
    b_fc = np.asarray(b_fc, dtype=np.float32)

    Bfull, S, _ = x.shape
    B = Bfull // N_CORES
    key = (S, B)
    if key not in _NC_CACHE:
        _NC_CACHE[key] = build_program(S, B, num_devices=N_CORES)
    nc = _NC_CACHE[key]

    in_maps = []
    for ci in range(N_CORES):
        shard = x[ci * B : (ci + 1) * B]
        in_maps.append(
            prep_core_inputs(shard, w_ih, w_hh, b_ih, b_hh, w_attn, w_fc, b_fc)
        )
    res = bass_utils.run_bass_kernel_spmd(nc, in_maps, core_ids=list(range(N_CORES)))
    out = np.concatenate([res.results[ci]["y"] for ci in range(N_CORES)], axis=0)
    return out.astype(np.float32)
